# revision 6
# baseline (speedup 1.0000x reference)
"""DKT next-question BCE loss on 8 trn2 NeuronCores.

Data-parallel over the student axis: 32 students per core. Host-side
prep aligns pred[t] with batch[t+1] (the loss pairs step t's prediction
with step t+1's attempted question), flattens (student, step) into rows
and zero-pads to 6400 rows per core. On device, each 128-row group does
two fused multiply-reduce ops (scalar_tensor_tensor + accum_out) on the
vector engine:
  s1[r]  =  sum_q pred[r,q] * batch[r, q]       (correct-answer half)
  s2n[r] = -sum_q pred[r,q] * batch[r, Q+q]     (incorrect-answer half)
Because batch rows are one-hot * correctness, v = s1 + s2n is +prob if
the row was answered correctly, -prob if not, and 0 for padded/empty
rows — so p = |v|, a = [v>0], mask = [v!=0]. The BCE tail runs
per-iteration on tiny [128,G] stats so it overlaps the DMA stream; the
iteration schedule tapers (9x G=5, then 5x G=1) so almost no compute is
exposed after the last DMA. Per-partition partials return to the host,
which sums across partitions and cores (the all-reduce of the scalar
loss) and negates.
"""

import sys

import numpy as np

sys.path.insert(0, "/opt/trn_rl_repo")

import concourse.bacc as bacc
import concourse.mybir as mybir
import concourse.tile as tile
from concourse.bass_utils import run_bass_kernel_spmd

B, T, Q = 256, 200, 1024
NCORES = 8
BS = B // NCORES              # students per core
ROWS = BS * (T - 1)           # 6368 valid rows per core
RPAD = 6400                   # padded rows
SCHEDULE = [5] * 9 + [1] * 5  # 128-row groups per iteration (sum = 50)
NITER = len(SCHEDULE)

F32 = mybir.dt.float32
_cache: dict = {}


def _build():
    nc = bacc.Bacc("TRN2", target_bir_lowering=False, debug=False,
                   num_devices=NCORES)
    pred_h = nc.dram_tensor("pred", [RPAD, Q], F32, kind="ExternalInput")
    batch_h = nc.dram_tensor("batch", [RPAD, 2 * Q], F32, kind="ExternalInput")
    out_h = nc.dram_tensor("out", [128, 1], F32, kind="ExternalOutput")

    mult = mybir.AluOpType.mult
    add = mybir.AluOpType.add
    Ln = mybir.ActivationFunctionType.Ln
    Abs = mybir.ActivationFunctionType.Abs

    with tile.TileContext(nc) as tc:
        with tc.tile_pool(name="pred_p", bufs=3) as pp, \
             tc.tile_pool(name="batch_p", bufs=3) as bp, \
             tc.tile_pool(name="prod_p", bufs=2) as sp, \
             tc.tile_pool(name="tail_p", bufs=2) as tp, \
             tc.tile_pool(name="acc_p", bufs=1) as ac:
            lsum = ac.tile([128, NITER], F32)
            off = 0
            for i, G in enumerate(SCHEDULE):
                pt = pp.tile([128, G, Q], F32, tag="pt")
                bt = bp.tile([128, G, 2 * Q], F32, tag="bt")
                rows = slice(off, off + G * 128)
                off += G * 128
                # separate HWDGE rings: pred on the scalar engine's,
                # batch on sync's
                nc.scalar.dma_start(
                    out=pt[:],
                    in_=pred_h[rows, :].rearrange("(g p) q -> p g q", p=128))
                nc.sync.dma_start(
                    out=bt[:],
                    in_=batch_h[rows, :].rearrange("(g p) q -> p g q", p=128))
                s1 = tp.tile([128, G], F32, tag="s1")
                s2n = tp.tile([128, G], F32, tag="s2n")
                for g in range(G):
                    prod = sp.tile([128, Q], F32, tag="prod")
                    nc.vector.scalar_tensor_tensor(
                        out=prod[:], in0=pt[:, g, :], scalar=1.0,
                        in1=bt[:, g, 0:Q], op0=mult, op1=mult,
                        accum_out=s1[:, g:g + 1])
                    prod2 = sp.tile([128, Q], F32, tag="prod")
                    nc.vector.scalar_tensor_tensor(
                        out=prod2[:], in0=pt[:, g, :], scalar=-1.0,
                        in1=bt[:, g, Q:2 * Q], op0=mult, op1=mult,
                        accum_out=s2n[:, g:g + 1])

                # BCE tail for this iteration's G columns, overlapped
                # with the next iterations' DMA.
                v = tp.tile([128, G], F32, tag="v")
                nc.vector.tensor_add(v[:], s1[:], s2n[:])
                p = tp.tile([128, G], F32, tag="p")
                nc.scalar.activation(p[:], v[:], Abs)
                a = tp.tile([128, G], F32, tag="a")
                nc.vector.tensor_scalar(out=a[:], in0=v[:], scalar1=0.0,
                                        scalar2=None,
                                        op0=mybir.AluOpType.is_gt)
                mask = tp.tile([128, G], F32, tag="mask")
                nc.vector.tensor_scalar(out=mask[:], in0=v[:], scalar1=0.0,
                                        scalar2=None,
                                        op0=mybir.AluOpType.not_equal)
                # safe p: 0.5 where v == 0 so Ln stays finite
                eq = tp.tile([128, G], F32, tag="eq")
                nc.vector.tensor_scalar(out=eq[:], in0=v[:], scalar1=0.0,
                                        scalar2=None,
                                        op0=mybir.AluOpType.is_equal)
                half = tp.tile([128, G], F32, tag="half")
                nc.vector.tensor_scalar(out=half[:], in0=eq[:], scalar1=0.5,
                                        scalar2=None, op0=mult)
                spf = tp.tile([128, G], F32, tag="spf")
                nc.vector.tensor_add(spf[:], half[:], p[:])
                lp = tp.tile([128, G], F32, tag="lp")
                nc.scalar.activation(lp[:], spf[:], Ln)
                lq = tp.tile([128, G], F32, tag="lq")
                nc.scalar.activation(lq[:], spf[:], Ln, bias=1.0, scale=-1.0)
                # ll = a*lp + (1-a)*lq, then mask out empty rows
                d = tp.tile([128, G], F32, tag="d")
                nc.vector.tensor_sub(d[:], lp[:], lq[:])
                ad = tp.tile([128, G], F32, tag="ad")
                nc.vector.tensor_mul(ad[:], a[:], d[:])
                ll = tp.tile([128, G], F32, tag="ll")
                nc.vector.tensor_add(ll[:], lq[:], ad[:])
                llm = tp.tile([128, G], F32, tag="llm")
                nc.vector.tensor_mul(llm[:], ll[:], mask[:])
                nc.vector.tensor_reduce(out=lsum[:, i:i + 1], in_=llm[:],
                                        axis=mybir.AxisListType.X, op=add)

            part = ac.tile([128, 1], F32)
            nc.vector.tensor_reduce(out=part[:], in_=lsum[:],
                                    axis=mybir.AxisListType.X, op=add)
            nc.sync.dma_start(out=out_h[:], in_=part[:])

    nc.compile()
    return nc


def _get_nc():
    if "nc" not in _cache:
        _cache["nc"] = _build()
    return _cache["nc"]


def _in_maps(pred: np.ndarray, batch: np.ndarray) -> list[dict]:
    pred = np.asarray(pred, dtype=np.float32)
    batch = np.asarray(batch, dtype=np.float32)
    maps = []
    for c in range(NCORES):
        sl = slice(c * BS, (c + 1) * BS)
        pc = np.zeros((RPAD, Q), np.float32)
        pc[:ROWS] = pred[sl, :T - 1, :].reshape(ROWS, Q)
        bc = np.zeros((RPAD, 2 * Q), np.float32)
        bc[:ROWS] = batch[sl, 1:, :].reshape(ROWS, 2 * Q)
        maps.append({"pred": pc, "batch": bc})
    return maps


def _axon_reset():
    """Best-effort device reset: clears wedged NRT state on the terminal
    left by previously crashed runs. No-op if the axon .so is absent."""
    try:
        import ctypes

        import jax
        jax.devices()
        lib = ctypes.CDLL("/opt/axon/libaxon_pjrt.so")
        lib.axon_reset.restype = ctypes.c_int64
        lib.axon_reset()
    except Exception:
        pass


def _run(pred: np.ndarray, batch: np.ndarray, trace: bool = False):
    nc = _get_nc()
    _axon_reset()
    res = run_bass_kernel_spmd(nc, _in_maps(pred, batch),
                               list(range(NCORES)), trace=trace)
    total = np.sum([np.asarray(r["out"], np.float64).sum()
                    for r in res.results])
    loss = np.array([-total], dtype=np.float32)
    return loss, res


def kernel(pred: np.ndarray, batch: np.ndarray) -> np.ndarray:
    loss, _ = _run(pred, batch)
    return loss


# revision 8
# speedup vs baseline: 1.1361x; 1.1361x over previous
"""DKT next-question BCE loss on 8 trn2 NeuronCores.

Data-parallel over the student axis: 32 students per core. Host-side
prep aligns pred[t] with batch[t+1] (the loss pairs step t's prediction
with step t+1's attempted question), flattens (student, step) into rows
and zero-pads to 6400 rows per core. On device, each 128-row group does
two fused multiply-reduce ops (scalar_tensor_tensor + accum_out) on the
vector engine:
  s1[r]  =  sum_q pred[r,q] * batch[r, q]       (correct-answer half)
  s2n[r] = -sum_q pred[r,q] * batch[r, Q+q]     (incorrect-answer half)
Because batch rows are one-hot * correctness, v = s1 + s2n is +prob if
the row was answered correctly, -prob if not, and 0 for padded/empty
rows — so p = |v|, a = [v>0], mask = [v!=0]. The BCE tail runs
per-iteration on tiny [128,G] stats so it overlaps the DMA stream; the
iteration schedule tapers (9x G=5, then 5x G=1) so almost no compute is
exposed after the last DMA. Per-partition partials return to the host,
which sums across partitions and cores (the all-reduce of the scalar
loss) and negates.
"""

import sys

import numpy as np

sys.path.insert(0, "/opt/trn_rl_repo")

import concourse.bacc as bacc
import concourse.mybir as mybir
import concourse.tile as tile
from concourse.bass_utils import run_bass_kernel_spmd

B, T, Q = 256, 200, 1024
NCORES = 8
BS = B // NCORES              # students per core
ROWS = BS * (T - 1)           # 6368 valid rows per core
RPAD = 6400                   # padded rows
# Each partition covers 2 adjacent DRAM rows (8KB/16KB descriptors);
# one "group" = 256 rows. Schedule tapers so the final iterations leave
# almost no compute exposed after the last DMA.
SCHEDULE = [2] * 10 + [1] * 5  # 256-row groups per iteration (sum = 25)
NITER = len(SCHEDULE)

F32 = mybir.dt.float32
_cache: dict = {}


def _build():
    nc = bacc.Bacc("TRN2", target_bir_lowering=False, debug=False,
                   num_devices=NCORES)
    pred_h = nc.dram_tensor("pred", [RPAD, Q], F32, kind="ExternalInput")
    batch_h = nc.dram_tensor("batch", [RPAD, 2 * Q], F32, kind="ExternalInput")
    out_h = nc.dram_tensor("out", [128, 1], F32, kind="ExternalOutput")

    mult = mybir.AluOpType.mult
    add = mybir.AluOpType.add
    Ln = mybir.ActivationFunctionType.Ln
    Abs = mybir.ActivationFunctionType.Abs

    with tile.TileContext(nc) as tc:
        with tc.tile_pool(name="pred_p", bufs=3) as pp, \
             tc.tile_pool(name="batch_p", bufs=3) as bp, \
             tc.tile_pool(name="prod_p", bufs=2) as sp, \
             tc.tile_pool(name="tail_p", bufs=2) as tp, \
             tc.tile_pool(name="acc_p", bufs=1) as ac:
            lsum = ac.tile([128, NITER], F32)
            off = 0
            for i, G in enumerate(SCHEDULE):
                NC_ = 2 * G  # stat columns this iteration (one per row)
                pt = pp.tile([128, G, 2, Q], F32, tag="pt")
                bt = bp.tile([128, G, 2, 2 * Q], F32, tag="bt")
                rows = slice(off, off + G * 256)
                off += G * 256
                # separate HWDGE rings: pred on the scalar engine's,
                # batch on sync's
                nc.scalar.dma_start(
                    out=pt[:],
                    in_=pred_h[rows, :].rearrange("(g p h) q -> p g h q",
                                                  p=128, h=2))
                nc.sync.dma_start(
                    out=bt[:],
                    in_=batch_h[rows, :].rearrange("(g p h) q -> p g h q",
                                                   p=128, h=2))
                s1 = tp.tile([128, NC_], F32, tag="s1")
                s2n = tp.tile([128, NC_], F32, tag="s2n")
                for g in range(G):
                    for h in range(2):
                        k = 2 * g + h
                        prod = sp.tile([128, Q], F32, tag="prod")
                        nc.vector.scalar_tensor_tensor(
                            out=prod[:], in0=pt[:, g, h, :], scalar=1.0,
                            in1=bt[:, g, h, 0:Q], op0=mult, op1=mult,
                            accum_out=s1[:, k:k + 1])
                        prod2 = sp.tile([128, Q], F32, tag="prod")
                        nc.vector.scalar_tensor_tensor(
                            out=prod2[:], in0=pt[:, g, h, :], scalar=-1.0,
                            in1=bt[:, g, h, Q:2 * Q], op0=mult, op1=mult,
                            accum_out=s2n[:, k:k + 1])

                # BCE tail for this iteration's columns, overlapped
                # with the next iterations' DMA.
                G = NC_
                v = tp.tile([128, G], F32, tag="v")
                nc.vector.tensor_add(v[:], s1[:], s2n[:])
                p = tp.tile([128, G], F32, tag="p")
                nc.scalar.activation(p[:], v[:], Abs)
                a = tp.tile([128, G], F32, tag="a")
                nc.vector.tensor_scalar(out=a[:], in0=v[:], scalar1=0.0,
                                        scalar2=None,
                                        op0=mybir.AluOpType.is_gt)
                mask = tp.tile([128, G], F32, tag="mask")
                nc.vector.tensor_scalar(out=mask[:], in0=v[:], scalar1=0.0,
                                        scalar2=None,
                                        op0=mybir.AluOpType.not_equal)
                # safe p: 0.5 where v == 0 so Ln stays finite
                eq = tp.tile([128, G], F32, tag="eq")
                nc.vector.tensor_scalar(out=eq[:], in0=v[:], scalar1=0.0,
                                        scalar2=None,
                                        op0=mybir.AluOpType.is_equal)
                half = tp.tile([128, G], F32, tag="half")
                nc.vector.tensor_scalar(out=half[:], in0=eq[:], scalar1=0.5,
                                        scalar2=None, op0=mult)
                spf = tp.tile([128, G], F32, tag="spf")
                nc.vector.tensor_add(spf[:], half[:], p[:])
                lp = tp.tile([128, G], F32, tag="lp")
                nc.scalar.activation(lp[:], spf[:], Ln)
                lq = tp.tile([128, G], F32, tag="lq")
                nc.scalar.activation(lq[:], spf[:], Ln, bias=1.0, scale=-1.0)
                # ll = a*lp + (1-a)*lq, then mask out empty rows
                d = tp.tile([128, G], F32, tag="d")
                nc.vector.tensor_sub(d[:], lp[:], lq[:])
                ad = tp.tile([128, G], F32, tag="ad")
                nc.vector.tensor_mul(ad[:], a[:], d[:])
                ll = tp.tile([128, G], F32, tag="ll")
                nc.vector.tensor_add(ll[:], lq[:], ad[:])
                llm = tp.tile([128, G], F32, tag="llm")
                nc.vector.tensor_mul(llm[:], ll[:], mask[:])
                nc.vector.tensor_reduce(out=lsum[:, i:i + 1], in_=llm[:],
                                        axis=mybir.AxisListType.X, op=add)

            part = ac.tile([128, 1], F32)
            nc.vector.tensor_reduce(out=part[:], in_=lsum[:],
                                    axis=mybir.AxisListType.X, op=add)
            nc.sync.dma_start(out=out_h[:], in_=part[:])

    nc.compile()
    return nc


def _get_nc():
    if "nc" not in _cache:
        _cache["nc"] = _build()
    return _cache["nc"]


def _in_maps(pred: np.ndarray, batch: np.ndarray) -> list[dict]:
    pred = np.asarray(pred, dtype=np.float32)
    batch = np.asarray(batch, dtype=np.float32)
    maps = []
    for c in range(NCORES):
        sl = slice(c * BS, (c + 1) * BS)
        pc = np.zeros((RPAD, Q), np.float32)
        pc[:ROWS] = pred[sl, :T - 1, :].reshape(ROWS, Q)
        bc = np.zeros((RPAD, 2 * Q), np.float32)
        bc[:ROWS] = batch[sl, 1:, :].reshape(ROWS, 2 * Q)
        maps.append({"pred": pc, "batch": bc})
    return maps


def _axon_reset():
    """Best-effort device reset: clears wedged NRT state on the terminal
    left by previously crashed runs. No-op if the axon .so is absent."""
    try:
        import ctypes

        import jax
        jax.devices()
        lib = ctypes.CDLL("/opt/axon/libaxon_pjrt.so")
        lib.axon_reset.restype = ctypes.c_int64
        lib.axon_reset()
    except Exception:
        pass


def _run(pred: np.ndarray, batch: np.ndarray, trace: bool = False):
    nc = _get_nc()
    _axon_reset()
    res = run_bass_kernel_spmd(nc, _in_maps(pred, batch),
                               list(range(NCORES)), trace=trace)
    total = np.sum([np.asarray(r["out"], np.float64).sum()
                    for r in res.results])
    loss = np.array([-total], dtype=np.float32)
    return loss, res


def kernel(pred: np.ndarray, batch: np.ndarray) -> np.ndarray:
    loss, _ = _run(pred, batch)
    return loss
